# revision 26
# baseline (speedup 1.0000x reference)
"""Trainium2 Bass kernel for nn_CrossMultiheadAttention_44074954391814.

v4 configuration (measured 185734 ns): head-sharded 2 heads/core, AV
batch-paired M=128, PE transposes, softmax adds mostly on DVE (8 on
gpsimd), sync-queue DMAs with y/x first-quarter column-halves.
"""

import sys

sys.path.insert(0, "/opt/trn_rl_repo")

from contextlib import ExitStack

import numpy as np

import concourse.bass as bass
import concourse.tile as tile
from concourse import bacc, mybir
from concourse.bass import ts
from concourse.bass_utils import run_bass_kernel_spmd
from concourse.masks import make_identity

D = 1024          # d_model
HEADS = 16
HD = 64           # head dim
B = 4
S = 1024
BS = B * S        # 4096
NCORES = 8
C = 128           # channels per core (2 heads * 64)
KT = D // 128     # 8 contraction tiles
FP16 = mybir.dt.float16
FP32 = mybir.dt.float32
SCALE = 1.0 / 8.0  # 1/sqrt(HD)
N_WARMUP = 10


def build_program():
    nc = bacc.Bacc("TRN2", target_bir_lowering=False, debug=False)

    yT = nc.dram_tensor("yT", [D, BS], FP16, kind="ExternalInput").ap()
    xT = nc.dram_tensor("xT", [D, BS], FP16, kind="ExternalInput").ap()
    wqkvT = nc.dram_tensor("wqkvT", [D, 3 * C], FP16, kind="ExternalInput").ap()
    woT = nc.dram_tensor("woT", [C, D], FP16, kind="ExternalInput").ap()
    bqkv = nc.dram_tensor("bqkv", [C, 3], FP32, kind="ExternalInput").ap()
    out = nc.dram_tensor("out", [BS, D], FP16, kind="ExternalOutput").ap()

    with tile.TileContext(nc) as tc, ExitStack() as ctx:
        consts = ctx.enter_context(tc.tile_pool(name="consts", bufs=1))
        qk = ctx.enter_context(tc.tile_pool(name="qk", bufs=1))
        vpool = ctx.enter_context(tc.tile_pool(name="vpool", bufs=1))
        apool = ctx.enter_context(tc.tile_pool(name="apool", bufs=1))
        atpool = ctx.enter_context(tc.tile_pool(name="atpool", bufs=1))

        ident = consts.tile([128, 128], FP16, tag="ident")
        make_identity(nc, ident)

        wdummy = consts.tile([128, 512], FP16, tag="wdummy")
        nc.gpsimd.memset(wdummy, 0.0)

        wqkv_sb = consts.tile([128, KT, 3 * C], FP16, tag="wqkv")
        wo_sb = consts.tile([C, D], FP16, tag="wo")
        bqkv_sb = consts.tile([C, 3], FP32, tag="bqkv")
        nc.sync.dma_start(
            out=wqkv_sb, in_=wqkvT.rearrange("(kt p) c -> p kt c", p=128)
        )

        qT = qk.tile([C, BS], FP16, tag="qT")
        kT = qk.tile([C, BS], FP16, tag="kT")
        vT = qk.tile([C, BS], FP16, tag="vT")
        vpair = vpool.tile([128, 2, 2, 8, 128], FP16, tag="vpair")

        A = apool.tile([128, 2, S // 128, S], FP16, tag="A")
        AT = atpool.tile([128, 2, S // 128, S], FP16, tag="AT")

        with (
            tc.tile_pool(name="xy", bufs=3) as xy,
            tc.tile_pool(name="pp_qkv", bufs=2, space="PSUM") as pp_qkv,
            tc.tile_pool(name="tp", bufs=2, space="PSUM") as tp,
            tc.tile_pool(name="pp_sc", bufs=2, space="PSUM") as pp_sc,
            tc.tile_pool(name="ppool", bufs=6) as ppool,
            tc.tile_pool(name="rpool", bufs=12) as rpool,
            tc.tile_pool(name="ovpool", bufs=4) as ovpool,
            tc.tile_pool(name="opool", bufs=4) as opool,
        ):
            def load_quarter(src_dram, g, tag, halves=1):
                q = xy.tile([128, KT, 1024], FP16, tag=tag, name=f"xy_{tag}_{g}")
                for hh in range(halves):
                    w = 1024 // halves
                    nc.sync.dma_start(
                        out=q[:, :, hh * w : (hh + 1) * w],
                        in_=src_dram[
                            :, g * 1024 + hh * w : g * 1024 + (hh + 1) * w
                        ].rearrange("(kt p) s -> p kt s", p=128),
                    )
                return q

            wps = pp_qkv.tile([128, 512], FP32, tag="ps", name="wps")
            for _ in range(N_WARMUP):
                nc.tensor.matmul(
                    wps, lhsT=wdummy[:, 0:128], rhs=wdummy, start=True, stop=True
                )

            def proj_group(src_q, wi, dst, g, n2):
                ps = pp_qkv.tile([C, 512], FP32, tag="ps", name="ps")
                for kt in range(KT):
                    nc.tensor.matmul(
                        ps,
                        lhsT=wqkv_sb[:, kt, wi * C : (wi + 1) * C],
                        rhs=src_q[:, kt, ts(n2, 512)],
                        start=(kt == 0),
                        stop=(kt == KT - 1),
                    )
                nc.vector.tensor_scalar_add(
                    out=dst[:, ts(g * 2 + n2, 512)],
                    in0=ps,
                    scalar1=bqkv_sb[:, wi : wi + 1],
                )

            def vtrans_group(g):
                vps = tp.tile([128, 1024], FP16, tag="tp", name="vps")
                for k in range(8):
                    nc.tensor.matmul(
                        vps[:, ts(k, 128)],
                        lhsT=vT[:, ts(g * 8 + k, 128)],
                        rhs=ident,
                        is_transpose=True,
                        start=(k == 0),
                        stop=(k == 7),
                    )
                vps3 = vps.rearrange("p (jt c) -> p jt c", jt=8)
                for h in range(2):
                    nc.vector.tensor_copy(
                        vpair[:, h, g // 2, :, (g % 2) * 64 : (g % 2) * 64 + 64],
                        vps3[:, :, h * 64 : h * 64 + 64],
                    )

            yq = xy.tile([128, KT, 1024], FP16, tag="xyq", name="xy_y0")
            xq = xy.tile([128, KT, 1024], FP16, tag="xyq", name="xy_x0")

            def load_half(q, src_dram, hh):
                nc.sync.dma_start(
                    out=q[:, :, hh * 512 : (hh + 1) * 512],
                    in_=src_dram[:, hh * 512 : (hh + 1) * 512].rearrange(
                        "(kt p) s -> p kt s", p=128
                    ),
                )

            nc.sync.dma_start(out=bqkv_sb, in_=bqkv)
            load_half(xq, xT, 0)
            load_half(yq, yT, 0)
            load_half(xq, xT, 1)
            load_half(yq, yT, 1)
            # chase the DMA halves: k needs all of x, scores also need
            # q n2=0; interleave so each proj starts as its half lands
            proj_group(xq, 1, kT, 0, 0)
            proj_group(yq, 0, qT, 0, 0)
            proj_group(xq, 1, kT, 0, 1)
            proj_group(yq, 0, qT, 0, 1)

            def warm(n):
                # dummy matmuls: keep the HAM clock gate open through the
                # transpose-heavy b=3 stretch (PE transposes don't count
                # as activity and the clock drops to 1.2 GHz otherwise)
                wp = pp_qkv.tile([128, 512], FP32, tag="ps", name="warm")
                for _ in range(n):
                    nc.tensor.matmul(
                        wp, lhsT=wdummy[:, 0:128], rhs=wdummy,
                        start=True, stop=True,
                    )

            def softmax_block(b, h, it):
                sc = pp_sc.tile([128, S], FP32, tag="sc", name="sc")
                for jt in range(2):
                    nc.tensor.matmul(
                        sc[:, ts(jt, 512)],
                        lhsT=qT[
                            h * 64 : h * 64 + 64,
                            b * S + it * 128 : b * S + (it + 1) * 128,
                        ],
                        rhs=kT[
                            h * 64 : h * 64 + 64,
                            b * S + jt * 512 : b * S + (jt + 1) * 512,
                        ],
                        start=True,
                        stop=True,
                    )
                P = ppool.tile([128, S], FP16, tag="P")
                r = rpool.tile([128, 1], FP32, tag="r")
                rinv = rpool.tile([128, 1], FP32, tag="rinv")
                nc.scalar.activation(
                    out=P,
                    in_=sc,
                    func=mybir.ActivationFunctionType.Exp,
                    scale=SCALE,
                    accum_out=r,
                )
                nc.vector.reciprocal(out=rinv, in_=r)
                if b == 0:
                    nc.vector.tensor_scalar_mul(
                        out=A[:, h, it, :], in0=P, scalar1=rinv
                    )
                else:
                    Pw = ppool.tile([128, S], FP16, tag="Pw", bufs=8)
                    nc.vector.tensor_scalar_mul(out=Pw, in0=P, scalar1=rinv)
                    if b == 1 and it % 2 == 0:
                        nc.gpsimd.tensor_add(A[:, h, it, :], A[:, h, it, :], Pw)
                    else:
                        nc.vector.tensor_add(A[:, h, it, :], A[:, h, it, :], Pw)

            def atrans_block(h, it, evac):
                aps = tp.tile([128, 1024], FP16, tag="tp", name="aps")
                for jt in range(8):
                    nc.tensor.matmul(
                        aps[:, ts(jt, 128)],
                        lhsT=A[:, h, it, ts(jt, 128)],
                        rhs=ident,
                        is_transpose=True,
                        start=(jt == 0),
                        stop=(jt == 7),
                    )
                aps3 = aps.rearrange("c (jt p) -> c jt p", jt=8)
                if evac == "v":
                    nc.vector.tensor_copy(AT[:, h, :, ts(it, 128)], aps3)
                else:
                    nc.scalar.copy(AT[:, h, :, ts(it, 128)], aps3)

            ovT = [
                ovpool.tile([C, S], FP16, tag="ovT", name=f"ovT{b}")
                for b in range(B)
            ]

            def av_chunk(h, p, n):
                av = pp_qkv.tile([128, 512], FP32, tag="ps", name="av")
                for jt in range(8):
                    nc.tensor.matmul(
                        av,
                        lhsT=vpair[:, h, p, jt, :],
                        rhs=AT[:, h, jt, ts(n, 512)],
                        start=(jt == 0),
                        stop=(jt == 7),
                    )
                for half in range(2):
                    b = 2 * p + half
                    dst = ovT[b][h * 64 : h * 64 + 64, ts(n, 512)]
                    src = av[half * 64 : half * 64 + 64, :]
                    if half == 0:
                        nc.scalar.copy(dst, src)
                    else:
                        nc.vector.tensor_copy(dst, src)

            def outproj_chunk(b, st):
                o_ps = pp_sc.tile([128, D], FP32, tag="sc", name="ops")
                for n in range(2):
                    nc.tensor.matmul(
                        o_ps[:, ts(n, 512)],
                        lhsT=ovT[b][:, ts(st, 128)],
                        rhs=wo_sb[:, ts(n, 512)],
                        start=True,
                        stop=True,
                    )
                o_sb = opool.tile([128, D], FP16, tag="osb")
                if st % 2 == 0:
                    nc.vector.tensor_copy(o_sb, o_ps)
                else:
                    nc.scalar.copy(o_sb, o_ps)
                nc.sync.dma_start(
                    out=out[b * S + st * 128 : b * S + (st + 1) * 128, :],
                    in_=o_sb,
                )

            for b in range(B):
                side = []
                if b == 0:
                    def load_wo():
                        nc.sync.dma_start(out=wo_sb, in_=woT)
                    side.append(load_wo)
                if b > 0:
                    # this quarter's v-proj first: its data is resident, so
                    # the PE never head-of-line blocks on the fresh quarter
                    # b+1 DMA at early its (the batch-boundary HAM stall)
                    for n2 in range(2):
                        side.append(
                            lambda n2=n2, xq=xq, b=b: proj_group(
                                xq, 2, vT, b, n2
                            )
                        )
                    side.append(lambda b=b: vtrans_group(b))
                if b < B - 1:
                    yq2 = load_quarter(yT, b + 1, "xyq")
                    xq2 = load_quarter(xT, b + 1, "xyq")
                    for n2 in range(2):
                        side.append(
                            lambda n2=n2, yq2=yq2, b=b: proj_group(
                                yq2, 0, qT, b + 1, n2
                            )
                        )
                    for n2 in range(2):
                        side.append(
                            lambda n2=n2, xq2=xq2, b=b: proj_group(
                                xq2, 1, kT, b + 1, n2
                            )
                        )
                if b == 0:
                    for n2 in range(2):
                        side.append(
                            lambda n2=n2, xq=xq, b=b: proj_group(
                                xq, 2, vT, b, n2
                            )
                        )
                    side.append(lambda b=b: vtrans_group(b))

                if b < B - 1:
                    for it in range(S // 128):
                        if it < len(side):
                            side[it]()
                        for h in range(2):
                            softmax_block(b, h, it)
                    xq = xq2
                else:
                    for it in range(S // 128):
                        if it < len(side):
                            side[it]()
                        softmax_block(b, 0, it)
                        if it >= 2:
                            atrans_block(0, it - 2, "v" if it % 2 else "s")
                    for it in range(S // 128):
                        softmax_block(b, 1, it)
                        if it == 0:
                            atrans_block(0, 6, "s")
                        elif it == 1:
                            atrans_block(0, 7, "v")
                        else:
                            atrans_block(1, it - 2, "v" if it % 2 else "s")
                        if it == 1:
                            av_chunk(0, 0, 0)
                        elif it == 3:
                            av_chunk(0, 0, 1)
                        elif it == 5:
                            av_chunk(0, 1, 0)
                        elif it == 7:
                            av_chunk(0, 1, 1)
                    atrans_block(1, 6, "s")
                    atrans_block(1, 7, "v")

            av_chunk(1, 0, 0)
            av_chunk(1, 0, 1)
            av_chunk(1, 1, 0)
            for st in range(4):
                outproj_chunk(0, st)
            av_chunk(1, 1, 1)
            for st in range(4, 8):
                outproj_chunk(0, st)
            for b in range(1, B):
                for st in range(8):
                    outproj_chunk(b, st)

    return nc


_PROGRAM = None


def _get_program():
    global _PROGRAM
    if _PROGRAM is None:
        _PROGRAM = build_program()
        _PROGRAM.finalize()
    return _PROGRAM


def _host_in_maps(x, y, Wq, Wk, Wv, Wo, bq, bk, bv):
    xT16 = np.ascontiguousarray(x.reshape(BS, D).T).astype(np.float16)
    yT16 = np.ascontiguousarray(y.reshape(BS, D).T).astype(np.float16)
    in_maps = []
    for c in range(NCORES):
        rows = slice(c * C, (c + 1) * C)
        wqkv = np.concatenate(
            [Wq[rows, :].T, Wk[rows, :].T, Wv[rows, :].T], axis=1
        )
        bqkv = np.stack([bq[rows], bk[rows], bv[rows]], axis=1)
        in_maps.append(
            {
                "yT": yT16,
                "xT": xT16,
                "wqkvT": np.ascontiguousarray(wqkv).astype(np.float16),
                "woT": np.ascontiguousarray(Wo[:, rows].T).astype(np.float16),
                "bqkv": np.ascontiguousarray(bqkv).astype(np.float32),
            }
        )
    return in_maps


def kernel(**inputs):
    x = np.asarray(inputs["x"], dtype=np.float32)
    y = np.asarray(inputs["y"], dtype=np.float32)
    Wq = np.asarray(inputs["Wq"], dtype=np.float32)
    Wk = np.asarray(inputs["Wk"], dtype=np.float32)
    Wv = np.asarray(inputs["Wv"], dtype=np.float32)
    Wo = np.asarray(inputs["Wo"], dtype=np.float32)
    bq = np.asarray(inputs["bq"], dtype=np.float32)
    bk = np.asarray(inputs["bk"], dtype=np.float32)
    bv = np.asarray(inputs["bv"], dtype=np.float32)
    bo = np.asarray(inputs["bo"], dtype=np.float32)

    in_maps = _host_in_maps(x, y, Wq, Wk, Wv, Wo, bq, bk, bv)
    nc = _get_program()
    res = run_bass_kernel_spmd(nc, in_maps, list(range(NCORES)))

    acc = np.zeros((BS, D), dtype=np.float32)
    for c in range(NCORES):
        acc += res.results[c]["out"].astype(np.float32)
    acc += bo[None, :]
    return acc.reshape(B, S, D)


# revision 27
# speedup vs baseline: 1.0235x; 1.0235x over previous
"""Trainium2 Bass kernel for nn_CrossMultiheadAttention_44074954391814.

v4 configuration (measured 185734 ns): head-sharded 2 heads/core, AV
batch-paired M=128, PE transposes, softmax adds mostly on DVE (8 on
gpsimd), sync-queue DMAs with y/x first-quarter column-halves.
"""

import sys

sys.path.insert(0, "/opt/trn_rl_repo")

from contextlib import ExitStack

import numpy as np

import concourse.bass as bass
import concourse.tile as tile
from concourse import bacc, mybir
from concourse.bass import ts
from concourse.bass_utils import run_bass_kernel_spmd
from concourse.masks import make_identity

D = 1024          # d_model
HEADS = 16
HD = 64           # head dim
B = 4
S = 1024
BS = B * S        # 4096
NCORES = 8
C = 128           # channels per core (2 heads * 64)
KT = D // 128     # 8 contraction tiles
FP16 = mybir.dt.float16
FP32 = mybir.dt.float32
SCALE = 1.0 / 8.0  # 1/sqrt(HD)
N_WARMUP = 10


def build_program():
    nc = bacc.Bacc("TRN2", target_bir_lowering=False, debug=False)

    yT = nc.dram_tensor("yT", [D, BS], FP16, kind="ExternalInput").ap()
    xT = nc.dram_tensor("xT", [D, BS], FP16, kind="ExternalInput").ap()
    wqkvT = nc.dram_tensor("wqkvT", [D, 3 * C], FP16, kind="ExternalInput").ap()
    woT = nc.dram_tensor("woT", [C, D], FP16, kind="ExternalInput").ap()
    bqkv = nc.dram_tensor("bqkv", [C, 3], FP32, kind="ExternalInput").ap()
    out = nc.dram_tensor("out", [BS, D], FP16, kind="ExternalOutput").ap()

    with tile.TileContext(nc) as tc, ExitStack() as ctx:
        consts = ctx.enter_context(tc.tile_pool(name="consts", bufs=1))
        qk = ctx.enter_context(tc.tile_pool(name="qk", bufs=1))
        vpool = ctx.enter_context(tc.tile_pool(name="vpool", bufs=1))
        apool = ctx.enter_context(tc.tile_pool(name="apool", bufs=1))
        atpool = ctx.enter_context(tc.tile_pool(name="atpool", bufs=1))

        ident = consts.tile([128, 128], FP16, tag="ident")
        make_identity(nc, ident)

        wdummy = consts.tile([128, 512], FP16, tag="wdummy")
        nc.gpsimd.memset(wdummy, 0.0)

        wqkv_sb = consts.tile([128, KT, 3 * C], FP16, tag="wqkv")
        wo_sb = consts.tile([C, D], FP16, tag="wo")
        bqkv_sb = consts.tile([C, 3], FP32, tag="bqkv")
        nc.sync.dma_start(
            out=wqkv_sb, in_=wqkvT.rearrange("(kt p) c -> p kt c", p=128)
        )

        qT = qk.tile([C, BS], FP16, tag="qT")
        kT = qk.tile([C, BS], FP16, tag="kT")
        vT = qk.tile([C, BS], FP16, tag="vT")
        vpair = vpool.tile([128, 2, 2, 8, 128], FP16, tag="vpair")

        A = apool.tile([128, 2, S // 128, S], FP16, tag="A")
        AT = atpool.tile([128, 2, S // 128, S], FP16, tag="AT")

        with (
            tc.tile_pool(name="xy", bufs=3) as xy,
            tc.tile_pool(name="pp_qkv", bufs=2, space="PSUM") as pp_qkv,
            tc.tile_pool(name="tp", bufs=2, space="PSUM") as tp,
            tc.tile_pool(name="pp_sc", bufs=2, space="PSUM") as pp_sc,
            tc.tile_pool(name="ppool", bufs=6) as ppool,
            tc.tile_pool(name="rpool", bufs=12) as rpool,
            tc.tile_pool(name="ovpool", bufs=4) as ovpool,
            tc.tile_pool(name="opool", bufs=4) as opool,
        ):
            def load_quarter(src_dram, g, tag, halves=1):
                q = xy.tile([128, KT, 1024], FP16, tag=tag, name=f"xy_{tag}_{g}")
                for hh in range(halves):
                    w = 1024 // halves
                    nc.sync.dma_start(
                        out=q[:, :, hh * w : (hh + 1) * w],
                        in_=src_dram[
                            :, g * 1024 + hh * w : g * 1024 + (hh + 1) * w
                        ].rearrange("(kt p) s -> p kt s", p=128),
                    )
                return q

            wps = pp_qkv.tile([128, 512], FP32, tag="ps", name="wps")
            for _ in range(N_WARMUP):
                nc.tensor.matmul(
                    wps, lhsT=wdummy[:, 0:128], rhs=wdummy, start=True, stop=True
                )

            def proj_group(src_q, wi, dst, g, n2):
                ps = pp_qkv.tile([C, 512], FP32, tag="ps", name="ps")
                for kt in range(KT):
                    nc.tensor.matmul(
                        ps,
                        lhsT=wqkv_sb[:, kt, wi * C : (wi + 1) * C],
                        rhs=src_q[:, kt, ts(n2, 512)],
                        start=(kt == 0),
                        stop=(kt == KT - 1),
                    )
                nc.vector.tensor_scalar_add(
                    out=dst[:, ts(g * 2 + n2, 512)],
                    in0=ps,
                    scalar1=bqkv_sb[:, wi : wi + 1],
                )

            def vtrans_group(g):
                vps = tp.tile([128, 1024], FP16, tag="tp", name="vps")
                for k in range(8):
                    nc.tensor.matmul(
                        vps[:, ts(k, 128)],
                        lhsT=vT[:, ts(g * 8 + k, 128)],
                        rhs=ident,
                        is_transpose=True,
                        start=(k == 0),
                        stop=(k == 7),
                    )
                vps3 = vps.rearrange("p (jt c) -> p jt c", jt=8)
                for h in range(2):
                    nc.vector.tensor_copy(
                        vpair[:, h, g // 2, :, (g % 2) * 64 : (g % 2) * 64 + 64],
                        vps3[:, :, h * 64 : h * 64 + 64],
                    )

            yq = xy.tile([128, KT, 1024], FP16, tag="xyq", name="xy_y0")
            xq = xy.tile([128, KT, 1024], FP16, tag="xyq", name="xy_x0")

            def load_half(q, src_dram, hh):
                nc.sync.dma_start(
                    out=q[:, :, hh * 512 : (hh + 1) * 512],
                    in_=src_dram[:, hh * 512 : (hh + 1) * 512].rearrange(
                        "(kt p) s -> p kt s", p=128
                    ),
                )

            nc.sync.dma_start(out=bqkv_sb, in_=bqkv)
            load_half(xq, xT, 0)
            load_half(yq, yT, 0)
            load_half(xq, xT, 1)
            load_half(yq, yT, 1)
            # chase the DMA halves: k needs all of x, scores also need
            # q n2=0; interleave so each proj starts as its half lands
            proj_group(xq, 1, kT, 0, 0)
            proj_group(yq, 0, qT, 0, 0)
            proj_group(xq, 1, kT, 0, 1)
            proj_group(yq, 0, qT, 0, 1)

            def warm(n):
                # dummy matmuls: keep the HAM clock gate open through the
                # transpose-heavy b=3 stretch (PE transposes don't count
                # as activity and the clock drops to 1.2 GHz otherwise)
                wp = pp_qkv.tile([128, 512], FP32, tag="ps", name="warm")
                for _ in range(n):
                    nc.tensor.matmul(
                        wp, lhsT=wdummy[:, 0:128], rhs=wdummy,
                        start=True, stop=True,
                    )

            def softmax_block(b, h, it):
                sc = pp_sc.tile([128, S], FP32, tag="sc", name="sc")
                for jt in range(2):
                    nc.tensor.matmul(
                        sc[:, ts(jt, 512)],
                        lhsT=qT[
                            h * 64 : h * 64 + 64,
                            b * S + it * 128 : b * S + (it + 1) * 128,
                        ],
                        rhs=kT[
                            h * 64 : h * 64 + 64,
                            b * S + jt * 512 : b * S + (jt + 1) * 512,
                        ],
                        start=True,
                        stop=True,
                    )
                P = ppool.tile([128, S], FP16, tag="P")
                r = rpool.tile([128, 1], FP32, tag="r")
                rinv = rpool.tile([128, 1], FP32, tag="rinv")
                nc.scalar.activation(
                    out=P,
                    in_=sc,
                    func=mybir.ActivationFunctionType.Exp,
                    scale=SCALE,
                    accum_out=r,
                )
                nc.vector.reciprocal(out=rinv, in_=r)
                if b == 0:
                    nc.vector.tensor_scalar_mul(
                        out=A[:, h, it, :], in0=P, scalar1=rinv
                    )
                else:
                    Pw = ppool.tile([128, S], FP16, tag="Pw", bufs=8)
                    nc.vector.tensor_scalar_mul(out=Pw, in0=P, scalar1=rinv)
                    if b == 1 and it % 2 == 0:
                        nc.gpsimd.tensor_add(A[:, h, it, :], A[:, h, it, :], Pw)
                    else:
                        nc.vector.tensor_add(A[:, h, it, :], A[:, h, it, :], Pw)

            def atrans_block(h, it, evac):
                aps = tp.tile([128, 1024], FP16, tag="tp", name="aps")
                for jt in range(8):
                    nc.tensor.matmul(
                        aps[:, ts(jt, 128)],
                        lhsT=A[:, h, it, ts(jt, 128)],
                        rhs=ident,
                        is_transpose=True,
                        start=(jt == 0),
                        stop=(jt == 7),
                    )
                aps3 = aps.rearrange("c (jt p) -> c jt p", jt=8)
                if evac == "v":
                    nc.vector.tensor_copy(AT[:, h, :, ts(it, 128)], aps3)
                else:
                    nc.scalar.copy(AT[:, h, :, ts(it, 128)], aps3)

            ovT = [
                ovpool.tile([C, S], FP16, tag="ovT", name=f"ovT{b}")
                for b in range(B)
            ]

            def av_chunk(h, p, n):
                av = pp_qkv.tile([128, 512], FP32, tag="ps", name="av")
                for jt in range(8):
                    nc.tensor.matmul(
                        av,
                        lhsT=vpair[:, h, p, jt, :],
                        rhs=AT[:, h, jt, ts(n, 512)],
                        start=(jt == 0),
                        stop=(jt == 7),
                    )
                for half in range(2):
                    b = 2 * p + half
                    dst = ovT[b][h * 64 : h * 64 + 64, ts(n, 512)]
                    src = av[half * 64 : half * 64 + 64, :]
                    if half == 0:
                        nc.scalar.copy(dst, src)
                    else:
                        nc.vector.tensor_copy(dst, src)

            def outproj_chunk(b, st):
                o_ps = pp_sc.tile([128, D], FP32, tag="sc", name="ops")
                for n in range(2):
                    nc.tensor.matmul(
                        o_ps[:, ts(n, 512)],
                        lhsT=ovT[b][:, ts(st, 128)],
                        rhs=wo_sb[:, ts(n, 512)],
                        start=True,
                        stop=True,
                    )
                o_sb = opool.tile([128, D], FP16, tag="osb")
                if st % 2 == 0:
                    nc.vector.tensor_copy(o_sb, o_ps)
                else:
                    nc.scalar.copy(o_sb, o_ps)
                nc.sync.dma_start(
                    out=out[b * S + st * 128 : b * S + (st + 1) * 128, :],
                    in_=o_sb,
                )

            for b in range(B):
                side = []
                if b == 0:
                    def load_wo():
                        nc.sync.dma_start(out=wo_sb, in_=woT)
                    side.append(load_wo)
                if b > 0:
                    # this quarter's v-proj first: its data is resident, so
                    # the PE never head-of-line blocks on the fresh quarter
                    # b+1 DMA at early its (the batch-boundary HAM stall)
                    for n2 in range(2):
                        side.append(
                            lambda n2=n2, xq=xq, b=b: proj_group(
                                xq, 2, vT, b, n2
                            )
                        )
                    side.append(lambda b=b: vtrans_group(b))
                if b < B - 1:
                    yq2 = load_quarter(yT, b + 1, "xyq")
                    xq2 = load_quarter(xT, b + 1, "xyq")
                    for n2 in range(2):
                        side.append(
                            lambda n2=n2, yq2=yq2, b=b: proj_group(
                                yq2, 0, qT, b + 1, n2
                            )
                        )
                    for n2 in range(2):
                        side.append(
                            lambda n2=n2, xq2=xq2, b=b: proj_group(
                                xq2, 1, kT, b + 1, n2
                            )
                        )
                if b == 0:
                    for n2 in range(2):
                        side.append(
                            lambda n2=n2, xq=xq, b=b: proj_group(
                                xq, 2, vT, b, n2
                            )
                        )
                    side.append(lambda b=b: vtrans_group(b))

                if b < B - 1:
                    for it in range(S // 128):
                        if it < len(side):
                            side[it]()
                        for h in range(2):
                            softmax_block(b, h, it)
                    xq = xq2
                else:
                    for it in range(S // 128):
                        if it < len(side):
                            side[it]()
                        softmax_block(b, 0, it)
                        if it >= 2:
                            atrans_block(0, it - 2, "v" if it % 2 else "s")
                    for it in range(S // 128):
                        softmax_block(b, 1, it)
                        if it == 0:
                            atrans_block(0, 6, "s")
                        elif it == 1:
                            atrans_block(0, 7, "v")
                        else:
                            atrans_block(1, it - 2, "v" if it % 2 else "s")
                        if it == 3:
                            av_chunk(0, 0, 0)
                        elif it == 5:
                            av_chunk(0, 0, 1)
                        elif it == 6:
                            av_chunk(0, 1, 0)
                        elif it == 7:
                            av_chunk(0, 1, 1)
                    atrans_block(1, 6, "s")
                    atrans_block(1, 7, "v")

            av_chunk(1, 0, 0)
            av_chunk(1, 0, 1)
            av_chunk(1, 1, 0)
            for st in range(4):
                outproj_chunk(0, st)
            av_chunk(1, 1, 1)
            for st in range(4, 8):
                outproj_chunk(0, st)
            for b in range(1, B):
                for st in range(8):
                    outproj_chunk(b, st)

    return nc


_PROGRAM = None


def _get_program():
    global _PROGRAM
    if _PROGRAM is None:
        _PROGRAM = build_program()
        _PROGRAM.finalize()
    return _PROGRAM


def _host_in_maps(x, y, Wq, Wk, Wv, Wo, bq, bk, bv):
    xT16 = np.ascontiguousarray(x.reshape(BS, D).T).astype(np.float16)
    yT16 = np.ascontiguousarray(y.reshape(BS, D).T).astype(np.float16)
    in_maps = []
    for c in range(NCORES):
        rows = slice(c * C, (c + 1) * C)
        wqkv = np.concatenate(
            [Wq[rows, :].T, Wk[rows, :].T, Wv[rows, :].T], axis=1
        )
        bqkv = np.stack([bq[rows], bk[rows], bv[rows]], axis=1)
        in_maps.append(
            {
                "yT": yT16,
                "xT": xT16,
                "wqkvT": np.ascontiguousarray(wqkv).astype(np.float16),
                "woT": np.ascontiguousarray(Wo[:, rows].T).astype(np.float16),
                "bqkv": np.ascontiguousarray(bqkv).astype(np.float32),
            }
        )
    return in_maps


def kernel(**inputs):
    x = np.asarray(inputs["x"], dtype=np.float32)
    y = np.asarray(inputs["y"], dtype=np.float32)
    Wq = np.asarray(inputs["Wq"], dtype=np.float32)
    Wk = np.asarray(inputs["Wk"], dtype=np.float32)
    Wv = np.asarray(inputs["Wv"], dtype=np.float32)
    Wo = np.asarray(inputs["Wo"], dtype=np.float32)
    bq = np.asarray(inputs["bq"], dtype=np.float32)
    bk = np.asarray(inputs["bk"], dtype=np.float32)
    bv = np.asarray(inputs["bv"], dtype=np.float32)
    bo = np.asarray(inputs["bo"], dtype=np.float32)

    in_maps = _host_in_maps(x, y, Wq, Wk, Wv, Wo, bq, bk, bv)
    nc = _get_program()
    res = run_bass_kernel_spmd(nc, in_maps, list(range(NCORES)))

    acc = np.zeros((BS, D), dtype=np.float32)
    for c in range(NCORES):
        acc += res.results[c]["out"].astype(np.float32)
    acc += bo[None, :]
    return acc.reshape(B, S, D)


# revision 28
# speedup vs baseline: 1.0239x; 1.0004x over previous
"""Trainium2 Bass kernel for nn_CrossMultiheadAttention_44074954391814.

Sharding: 16 heads / 8 cores = 2 heads per core (128 of 1024 channels).
The batch-sum of attention is per-head, so with head sharding it stays
local to a core - no collective needed.  Each core reads the full x,y
(transposed + fp16 on host) and emits a partial (B*S, D) output (its
128-channel slice of the Wo contraction); the host sums the 8 partials
and adds bo.

Schedule facts learned from ntff traces (see HAM clock gate notes):
 - PE runs at 1.2 GHz until ~3.4us of sustained matmul activity and
   re-throttles after idle windows; dummy warmup matmuls bridge the
   input-DMA window at the start.
 - K=64 score matmuls for the two heads (partitions 0-63 / 64-127)
   auto-pack into PE row groups and run concurrently.
 - The softmax stream is paced by ScalarE (exp + row-sum accumulator,
   ~1.4us per 128-row block); DVE does 1/r and the A accumulation (a
   few adds go to gpsimd); evacuations are split ScalarE/DVE.
 - AV is batch-paired: out[(b0 ch | b1 ch), q] = one M=128 matmul per
   k-tile, since both batches contract the same summed attention A.
 - b=3 runs h-outer with A transposes (PE+identity) lagged two blocks
   and AV(h=0) interleaved into the h=1 stream via the pp_qkv psum
   ring; out-proj chunks cycle the pp_sc ring.
"""

import sys

sys.path.insert(0, "/opt/trn_rl_repo")

from contextlib import ExitStack

import numpy as np

import concourse.bass as bass
import concourse.tile as tile
from concourse import bacc, mybir
from concourse.bass import ts
from concourse.bass_utils import run_bass_kernel_spmd
from concourse.masks import make_identity

D = 1024          # d_model
HEADS = 16
HD = 64           # head dim
B = 4
S = 1024
BS = B * S        # 4096
NCORES = 8
C = 128           # channels per core (2 heads * 64)
KT = D // 128     # 8 contraction tiles
FP16 = mybir.dt.float16
FP32 = mybir.dt.float32
SCALE = 1.0 / 8.0  # 1/sqrt(HD)
N_WARMUP = 10


def build_program():
    nc = bacc.Bacc("TRN2", target_bir_lowering=False, debug=False)

    yT = nc.dram_tensor("yT", [D, BS], FP16, kind="ExternalInput").ap()
    xT = nc.dram_tensor("xT", [D, BS], FP16, kind="ExternalInput").ap()
    wqkvT = nc.dram_tensor("wqkvT", [D, 3 * C], FP16, kind="ExternalInput").ap()
    woT = nc.dram_tensor("woT", [C, D], FP16, kind="ExternalInput").ap()
    bqkv = nc.dram_tensor("bqkv", [C, 3], FP32, kind="ExternalInput").ap()
    out = nc.dram_tensor("out", [BS, D], FP16, kind="ExternalOutput").ap()

    with tile.TileContext(nc) as tc, ExitStack() as ctx:
        consts = ctx.enter_context(tc.tile_pool(name="consts", bufs=1))
        qk = ctx.enter_context(tc.tile_pool(name="qk", bufs=1))
        vpool = ctx.enter_context(tc.tile_pool(name="vpool", bufs=1))
        apool = ctx.enter_context(tc.tile_pool(name="apool", bufs=1))
        atpool = ctx.enter_context(tc.tile_pool(name="atpool", bufs=1))

        ident = consts.tile([128, 128], FP16, tag="ident")
        make_identity(nc, ident)

        wdummy = consts.tile([128, 512], FP16, tag="wdummy")
        nc.gpsimd.memset(wdummy, 0.0)

        wqkv_sb = consts.tile([128, KT, 3 * C], FP16, tag="wqkv")
        wo_sb = consts.tile([C, D], FP16, tag="wo")
        bqkv_sb = consts.tile([C, 3], FP32, tag="bqkv")
        nc.sync.dma_start(
            out=wqkv_sb, in_=wqkvT.rearrange("(kt p) c -> p kt c", p=128)
        )

        qT = qk.tile([C, BS], FP16, tag="qT")
        kT = qk.tile([C, BS], FP16, tag="kT")
        vT = qk.tile([C, BS], FP16, tag="vT")
        vpair = vpool.tile([128, 2, 2, 8, 128], FP16, tag="vpair")

        A = apool.tile([128, 2, S // 128, S], FP16, tag="A")
        AT = atpool.tile([128, 2, S // 128, S], FP16, tag="AT")

        with (
            tc.tile_pool(name="xy", bufs=3) as xy,
            tc.tile_pool(name="pp_qkv", bufs=2, space="PSUM") as pp_qkv,
            tc.tile_pool(name="tp", bufs=2, space="PSUM") as tp,
            tc.tile_pool(name="pp_sc", bufs=2, space="PSUM") as pp_sc,
            tc.tile_pool(name="ppool", bufs=6) as ppool,
            tc.tile_pool(name="rpool", bufs=12) as rpool,
            tc.tile_pool(name="ovpool", bufs=4) as ovpool,
            tc.tile_pool(name="opool", bufs=4) as opool,
        ):
            def load_quarter(src_dram, g, tag, halves=1):
                q = xy.tile([128, KT, 1024], FP16, tag=tag, name=f"xy_{tag}_{g}")
                for hh in range(halves):
                    w = 1024 // halves
                    nc.sync.dma_start(
                        out=q[:, :, hh * w : (hh + 1) * w],
                        in_=src_dram[
                            :, g * 1024 + hh * w : g * 1024 + (hh + 1) * w
                        ].rearrange("(kt p) s -> p kt s", p=128),
                    )
                return q

            wps = pp_qkv.tile([128, 512], FP32, tag="ps", name="wps")
            for _ in range(N_WARMUP):
                nc.tensor.matmul(
                    wps, lhsT=wdummy[:, 0:128], rhs=wdummy, start=True, stop=True
                )

            def proj_group(src_q, wi, dst, g, n2):
                ps = pp_qkv.tile([C, 512], FP32, tag="ps", name="ps")
                for kt in range(KT):
                    nc.tensor.matmul(
                        ps,
                        lhsT=wqkv_sb[:, kt, wi * C : (wi + 1) * C],
                        rhs=src_q[:, kt, ts(n2, 512)],
                        start=(kt == 0),
                        stop=(kt == KT - 1),
                    )
                nc.vector.tensor_scalar_add(
                    out=dst[:, ts(g * 2 + n2, 512)],
                    in0=ps,
                    scalar1=bqkv_sb[:, wi : wi + 1],
                )

            def vtrans_group(g):
                vps = tp.tile([128, 1024], FP16, tag="tp", name="vps")
                for k in range(8):
                    nc.tensor.matmul(
                        vps[:, ts(k, 128)],
                        lhsT=vT[:, ts(g * 8 + k, 128)],
                        rhs=ident,
                        is_transpose=True,
                        start=(k == 0),
                        stop=(k == 7),
                    )
                vps3 = vps.rearrange("p (jt c) -> p jt c", jt=8)
                for h in range(2):
                    nc.vector.tensor_copy(
                        vpair[:, h, g // 2, :, (g % 2) * 64 : (g % 2) * 64 + 64],
                        vps3[:, :, h * 64 : h * 64 + 64],
                    )

            yq = xy.tile([128, KT, 1024], FP16, tag="xyq", name="xy_y0")
            xq = xy.tile([128, KT, 1024], FP16, tag="xyq", name="xy_x0")

            def load_half(q, src_dram, hh):
                nc.sync.dma_start(
                    out=q[:, :, hh * 512 : (hh + 1) * 512],
                    in_=src_dram[:, hh * 512 : (hh + 1) * 512].rearrange(
                        "(kt p) s -> p kt s", p=128
                    ),
                )

            nc.sync.dma_start(out=bqkv_sb, in_=bqkv)
            load_half(xq, xT, 0)
            load_half(yq, yT, 0)
            load_half(xq, xT, 1)
            load_half(yq, yT, 1)
            # chase the DMA halves: k needs all of x, scores also need
            # q n2=0; interleave so each proj starts as its half lands
            proj_group(xq, 1, kT, 0, 0)
            proj_group(yq, 0, qT, 0, 0)
            proj_group(xq, 1, kT, 0, 1)
            proj_group(yq, 0, qT, 0, 1)

            def warm(n):
                # dummy matmuls: keep the HAM clock gate open through the
                # transpose-heavy b=3 stretch (PE transposes don't count
                # as activity and the clock drops to 1.2 GHz otherwise)
                wp = pp_qkv.tile([128, 512], FP32, tag="ps", name="warm")
                for _ in range(n):
                    nc.tensor.matmul(
                        wp, lhsT=wdummy[:, 0:128], rhs=wdummy,
                        start=True, stop=True,
                    )

            def softmax_block(b, h, it):
                sc = pp_sc.tile([128, S], FP32, tag="sc", name="sc")
                for jt in range(2):
                    nc.tensor.matmul(
                        sc[:, ts(jt, 512)],
                        lhsT=qT[
                            h * 64 : h * 64 + 64,
                            b * S + it * 128 : b * S + (it + 1) * 128,
                        ],
                        rhs=kT[
                            h * 64 : h * 64 + 64,
                            b * S + jt * 512 : b * S + (jt + 1) * 512,
                        ],
                        start=True,
                        stop=True,
                    )
                P = ppool.tile([128, S], FP16, tag="P")
                r = rpool.tile([128, 1], FP32, tag="r")
                rinv = rpool.tile([128, 1], FP32, tag="rinv")
                nc.scalar.activation(
                    out=P,
                    in_=sc,
                    func=mybir.ActivationFunctionType.Exp,
                    scale=SCALE,
                    accum_out=r,
                )
                nc.vector.reciprocal(out=rinv, in_=r)
                if b == 0:
                    nc.vector.tensor_scalar_mul(
                        out=A[:, h, it, :], in0=P, scalar1=rinv
                    )
                else:
                    Pw = ppool.tile([128, S], FP16, tag="Pw", bufs=8)
                    nc.vector.tensor_scalar_mul(out=Pw, in0=P, scalar1=rinv)
                    if b == 1 and it % 2 == 0:
                        nc.gpsimd.tensor_add(A[:, h, it, :], A[:, h, it, :], Pw)
                    else:
                        nc.vector.tensor_add(A[:, h, it, :], A[:, h, it, :], Pw)

            def atrans_block(h, it, evac):
                aps = tp.tile([128, 1024], FP16, tag="tp", name="aps")
                for jt in range(8):
                    nc.tensor.matmul(
                        aps[:, ts(jt, 128)],
                        lhsT=A[:, h, it, ts(jt, 128)],
                        rhs=ident,
                        is_transpose=True,
                        start=(jt == 0),
                        stop=(jt == 7),
                    )
                aps3 = aps.rearrange("c (jt p) -> c jt p", jt=8)
                if evac == "v":
                    nc.vector.tensor_copy(AT[:, h, :, ts(it, 128)], aps3)
                else:
                    nc.scalar.copy(AT[:, h, :, ts(it, 128)], aps3)

            ovT = [
                ovpool.tile([C, S], FP16, tag="ovT", name=f"ovT{b}")
                for b in range(B)
            ]

            def av_chunk(h, p, n):
                av = pp_qkv.tile([128, 512], FP32, tag="ps", name="av")
                for jt in range(8):
                    nc.tensor.matmul(
                        av,
                        lhsT=vpair[:, h, p, jt, :],
                        rhs=AT[:, h, jt, ts(n, 512)],
                        start=(jt == 0),
                        stop=(jt == 7),
                    )
                for half in range(2):
                    b = 2 * p + half
                    dst = ovT[b][h * 64 : h * 64 + 64, ts(n, 512)]
                    src = av[half * 64 : half * 64 + 64, :]
                    if half == 0:
                        nc.scalar.copy(dst, src)
                    else:
                        nc.vector.tensor_copy(dst, src)

            def outproj_chunk(b, st):
                o_ps = pp_sc.tile([128, D], FP32, tag="sc", name="ops")
                for n in range(2):
                    nc.tensor.matmul(
                        o_ps[:, ts(n, 512)],
                        lhsT=ovT[b][:, ts(st, 128)],
                        rhs=wo_sb[:, ts(n, 512)],
                        start=True,
                        stop=True,
                    )
                o_sb = opool.tile([128, D], FP16, tag="osb")
                if st % 2 == 0:
                    nc.vector.tensor_copy(o_sb, o_ps)
                else:
                    nc.scalar.copy(o_sb, o_ps)
                nc.sync.dma_start(
                    out=out[b * S + st * 128 : b * S + (st + 1) * 128, :],
                    in_=o_sb,
                )

            for b in range(B):
                side = []
                if b == 0:
                    def load_wo():
                        nc.sync.dma_start(out=wo_sb, in_=woT)
                    side.append(load_wo)
                if b > 0:
                    # this quarter's v-proj first: its data is resident, so
                    # the PE never head-of-line blocks on the fresh quarter
                    # b+1 DMA at early its (the batch-boundary HAM stall)
                    for n2 in range(2):
                        side.append(
                            lambda n2=n2, xq=xq, b=b: proj_group(
                                xq, 2, vT, b, n2
                            )
                        )
                    side.append(lambda b=b: vtrans_group(b))
                if b < B - 1:
                    yq2 = load_quarter(yT, b + 1, "xyq")
                    xq2 = load_quarter(xT, b + 1, "xyq")
                    for n2 in range(2):
                        side.append(
                            lambda n2=n2, yq2=yq2, b=b: proj_group(
                                yq2, 0, qT, b + 1, n2
                            )
                        )
                    for n2 in range(2):
                        side.append(
                            lambda n2=n2, xq2=xq2, b=b: proj_group(
                                xq2, 1, kT, b + 1, n2
                            )
                        )
                if b == 0:
                    for n2 in range(2):
                        side.append(
                            lambda n2=n2, xq=xq, b=b: proj_group(
                                xq, 2, vT, b, n2
                            )
                        )
                    side.append(lambda b=b: vtrans_group(b))

                if b < B - 1:
                    for it in range(S // 128):
                        if it < len(side):
                            side[it]()
                        for h in range(2):
                            softmax_block(b, h, it)
                    xq = xq2
                else:
                    for it in range(S // 128):
                        if it < len(side):
                            side[it]()
                        softmax_block(b, 0, it)
                        if it >= 2:
                            atrans_block(0, it - 2, "v" if it % 2 else "s")
                    for it in range(S // 128):
                        softmax_block(b, 1, it)
                        if it == 0:
                            atrans_block(0, 6, "s")
                        elif it == 1:
                            atrans_block(0, 7, "v")
                        else:
                            atrans_block(1, it - 2, "v" if it % 2 else "s")
                        if it == 3:
                            av_chunk(0, 0, 0)
                        elif it == 5:
                            av_chunk(0, 0, 1)
                        elif it == 6:
                            av_chunk(0, 1, 0)
                        elif it == 7:
                            av_chunk(0, 1, 1)
                    atrans_block(1, 6, "s")
                    atrans_block(1, 7, "v")

            av_chunk(1, 0, 0)
            av_chunk(1, 0, 1)
            av_chunk(1, 1, 0)
            for st in range(4):
                outproj_chunk(0, st)
            av_chunk(1, 1, 1)
            for st in range(4, 8):
                outproj_chunk(0, st)
            for b in range(1, B):
                for st in range(8):
                    outproj_chunk(b, st)

    return nc


_PROGRAM = None


def _get_program():
    global _PROGRAM
    if _PROGRAM is None:
        _PROGRAM = build_program()
        _PROGRAM.finalize()
    return _PROGRAM


def _host_in_maps(x, y, Wq, Wk, Wv, Wo, bq, bk, bv):
    xT16 = np.ascontiguousarray(x.reshape(BS, D).T).astype(np.float16)
    yT16 = np.ascontiguousarray(y.reshape(BS, D).T).astype(np.float16)
    in_maps = []
    for c in range(NCORES):
        rows = slice(c * C, (c + 1) * C)
        wqkv = np.concatenate(
            [Wq[rows, :].T, Wk[rows, :].T, Wv[rows, :].T], axis=1
        )
        bqkv = np.stack([bq[rows], bk[rows], bv[rows]], axis=1)
        in_maps.append(
            {
                "yT": yT16,
                "xT": xT16,
                "wqkvT": np.ascontiguousarray(wqkv).astype(np.float16),
                "woT": np.ascontiguousarray(Wo[:, rows].T).astype(np.float16),
                "bqkv": np.ascontiguousarray(bqkv).astype(np.float32),
            }
        )
    return in_maps


def kernel(**inputs):
    x = np.asarray(inputs["x"], dtype=np.float32)
    y = np.asarray(inputs["y"], dtype=np.float32)
    Wq = np.asarray(inputs["Wq"], dtype=np.float32)
    Wk = np.asarray(inputs["Wk"], dtype=np.float32)
    Wv = np.asarray(inputs["Wv"], dtype=np.float32)
    Wo = np.asarray(inputs["Wo"], dtype=np.float32)
    bq = np.asarray(inputs["bq"], dtype=np.float32)
    bk = np.asarray(inputs["bk"], dtype=np.float32)
    bv = np.asarray(inputs["bv"], dtype=np.float32)
    bo = np.asarray(inputs["bo"], dtype=np.float32)

    in_maps = _host_in_maps(x, y, Wq, Wk, Wv, Wo, bq, bk, bv)
    nc = _get_program()
    res = run_bass_kernel_spmd(nc, in_maps, list(range(NCORES)))

    acc = np.zeros((BS, D), dtype=np.float32)
    for c in range(NCORES):
        acc += res.results[c]["out"].astype(np.float32)
    acc += bo[None, :]
    return acc.reshape(B, S, D)


# revision 29
# speedup vs baseline: 1.0268x; 1.0029x over previous
"""Trainium2 Bass kernel for nn_CrossMultiheadAttention_44074954391814.

Sharding: 16 heads / 8 cores = 2 heads per core (128 of 1024 channels).
The batch-sum of attention is per-head, so with head sharding it stays
local to a core - no collective needed.  Each core reads the full x,y
(transposed + fp16 on host) and emits a partial (B*S, D) output (its
128-channel slice of the Wo contraction); the host sums the 8 partials
and adds bo.

Schedule facts learned from ntff traces (see HAM clock gate notes):
 - PE runs at 1.2 GHz until ~3.4us of sustained matmul activity and
   re-throttles after idle windows; dummy warmup matmuls bridge the
   input-DMA window at the start.
 - K=64 score matmuls for the two heads (partitions 0-63 / 64-127)
   auto-pack into PE row groups and run concurrently.
 - The softmax stream is paced by ScalarE (exp + row-sum accumulator,
   ~1.4us per 128-row block); DVE does 1/r and the A accumulation (a
   few adds go to gpsimd); evacuations are split ScalarE/DVE.
 - AV is batch-paired: out[(b0 ch | b1 ch), q] = one M=128 matmul per
   k-tile, since both batches contract the same summed attention A.
 - b=3 runs h-outer with A transposes (PE+identity) lagged two blocks
   and AV(h=0) interleaved into the h=1 stream via the pp_qkv psum
   ring; out-proj chunks cycle the pp_sc ring.
"""

import sys

sys.path.insert(0, "/opt/trn_rl_repo")

from contextlib import ExitStack

import numpy as np

import concourse.bass as bass
import concourse.tile as tile
from concourse import bacc, mybir
from concourse.bass import ts
from concourse.bass_utils import run_bass_kernel_spmd
from concourse.masks import make_identity

D = 1024          # d_model
HEADS = 16
HD = 64           # head dim
B = 4
S = 1024
BS = B * S        # 4096
NCORES = 8
C = 128           # channels per core (2 heads * 64)
KT = D // 128     # 8 contraction tiles
FP16 = mybir.dt.float16
FP32 = mybir.dt.float32
SCALE = 1.0 / 8.0  # 1/sqrt(HD)
N_WARMUP = 10


def build_program():
    nc = bacc.Bacc("TRN2", target_bir_lowering=False, debug=False)

    yT = nc.dram_tensor("yT", [D, BS], FP16, kind="ExternalInput").ap()
    xT = nc.dram_tensor("xT", [D, BS], FP16, kind="ExternalInput").ap()
    wqkvT = nc.dram_tensor("wqkvT", [D, 3 * C], FP16, kind="ExternalInput").ap()
    woT = nc.dram_tensor("woT", [C, D], FP16, kind="ExternalInput").ap()
    bqkv = nc.dram_tensor("bqkv", [C, 3], FP32, kind="ExternalInput").ap()
    out = nc.dram_tensor("out", [BS, D], FP16, kind="ExternalOutput").ap()

    with tile.TileContext(nc) as tc, ExitStack() as ctx:
        consts = ctx.enter_context(tc.tile_pool(name="consts", bufs=1))
        qk = ctx.enter_context(tc.tile_pool(name="qk", bufs=1))
        vpool = ctx.enter_context(tc.tile_pool(name="vpool", bufs=1))
        apool = ctx.enter_context(tc.tile_pool(name="apool", bufs=1))
        atpool = ctx.enter_context(tc.tile_pool(name="atpool", bufs=1))

        ident = consts.tile([128, 128], FP16, tag="ident")
        make_identity(nc, ident)

        wdummy = consts.tile([128, 512], FP16, tag="wdummy")
        nc.gpsimd.memset(wdummy, 0.0)

        wqkv_sb = consts.tile([128, KT, 3 * C], FP16, tag="wqkv")
        wo_sb = consts.tile([C, D], FP16, tag="wo")
        bqkv_sb = consts.tile([C, 3], FP32, tag="bqkv")
        nc.sync.dma_start(
            out=wqkv_sb, in_=wqkvT.rearrange("(kt p) c -> p kt c", p=128)
        )

        qT = qk.tile([C, BS], FP16, tag="qT")
        kT = qk.tile([C, BS], FP16, tag="kT")
        vT = qk.tile([C, BS], FP16, tag="vT")
        vpair = vpool.tile([128, 2, 2, 8, 128], FP16, tag="vpair")

        A = apool.tile([128, 2, S // 128, S], FP16, tag="A")
        AT = atpool.tile([128, 2, S // 128, S], FP16, tag="AT")

        with (
            tc.tile_pool(name="xy", bufs=3) as xy,
            tc.tile_pool(name="pp_qkv", bufs=2, space="PSUM") as pp_qkv,
            tc.tile_pool(name="tp", bufs=2, space="PSUM") as tp,
            tc.tile_pool(name="pp_sc", bufs=2, space="PSUM") as pp_sc,
            tc.tile_pool(name="ppool", bufs=6) as ppool,
            tc.tile_pool(name="rpool", bufs=12) as rpool,
            tc.tile_pool(name="ovpool", bufs=4) as ovpool,
            tc.tile_pool(name="opool", bufs=4) as opool,
        ):
            def load_quarter(src_dram, g, tag, halves=1):
                q = xy.tile([128, KT, 1024], FP16, tag=tag, name=f"xy_{tag}_{g}")
                for hh in range(halves):
                    w = 1024 // halves
                    nc.sync.dma_start(
                        out=q[:, :, hh * w : (hh + 1) * w],
                        in_=src_dram[
                            :, g * 1024 + hh * w : g * 1024 + (hh + 1) * w
                        ].rearrange("(kt p) s -> p kt s", p=128),
                    )
                return q

            wps = pp_qkv.tile([128, 512], FP32, tag="ps", name="wps")
            for _ in range(N_WARMUP):
                nc.tensor.matmul(
                    wps, lhsT=wdummy[:, 0:128], rhs=wdummy, start=True, stop=True
                )

            def proj_group(src_q, wi, dst, g, n2):
                ps = pp_qkv.tile([C, 512], FP32, tag="ps", name="ps")
                for kt in range(KT):
                    nc.tensor.matmul(
                        ps,
                        lhsT=wqkv_sb[:, kt, wi * C : (wi + 1) * C],
                        rhs=src_q[:, kt, ts(n2, 512)],
                        start=(kt == 0),
                        stop=(kt == KT - 1),
                    )
                nc.vector.tensor_scalar_add(
                    out=dst[:, ts(g * 2 + n2, 512)],
                    in0=ps,
                    scalar1=bqkv_sb[:, wi : wi + 1],
                )

            def vtrans_group(g):
                vps = tp.tile([128, 1024], FP16, tag="tp", name="vps")
                for k in range(8):
                    nc.tensor.matmul(
                        vps[:, ts(k, 128)],
                        lhsT=vT[:, ts(g * 8 + k, 128)],
                        rhs=ident,
                        is_transpose=True,
                        start=(k == 0),
                        stop=(k == 7),
                    )
                vps3 = vps.rearrange("p (jt c) -> p jt c", jt=8)
                for h in range(2):
                    nc.vector.tensor_copy(
                        vpair[:, h, g // 2, :, (g % 2) * 64 : (g % 2) * 64 + 64],
                        vps3[:, :, h * 64 : h * 64 + 64],
                    )

            yq = xy.tile([128, KT, 1024], FP16, tag="xyq", name="xy_y0")
            xq = xy.tile([128, KT, 1024], FP16, tag="xyq", name="xy_x0")

            def load_half(q, src_dram, hh):
                nc.sync.dma_start(
                    out=q[:, :, hh * 512 : (hh + 1) * 512],
                    in_=src_dram[:, hh * 512 : (hh + 1) * 512].rearrange(
                        "(kt p) s -> p kt s", p=128
                    ),
                )

            nc.sync.dma_start(out=bqkv_sb, in_=bqkv)
            load_half(xq, xT, 0)
            load_half(yq, yT, 0)
            load_half(xq, xT, 1)
            load_half(yq, yT, 1)
            # chase the DMA halves: k needs all of x, scores also need
            # q n2=0; interleave so each proj starts as its half lands
            proj_group(xq, 1, kT, 0, 0)
            proj_group(yq, 0, qT, 0, 0)
            proj_group(xq, 1, kT, 0, 1)
            proj_group(yq, 0, qT, 0, 1)

            def warm(n):
                # dummy matmuls: keep the HAM clock gate open through the
                # transpose-heavy b=3 stretch (PE transposes don't count
                # as activity and the clock drops to 1.2 GHz otherwise)
                wp = pp_qkv.tile([128, 512], FP32, tag="ps", name="warm")
                for _ in range(n):
                    nc.tensor.matmul(
                        wp, lhsT=wdummy[:, 0:128], rhs=wdummy,
                        start=True, stop=True,
                    )

            def softmax_block(b, h, it):
                sc = pp_sc.tile([128, S], FP32, tag="sc", name="sc")
                for jt in range(2):
                    nc.tensor.matmul(
                        sc[:, ts(jt, 512)],
                        lhsT=qT[
                            h * 64 : h * 64 + 64,
                            b * S + it * 128 : b * S + (it + 1) * 128,
                        ],
                        rhs=kT[
                            h * 64 : h * 64 + 64,
                            b * S + jt * 512 : b * S + (jt + 1) * 512,
                        ],
                        start=True,
                        stop=True,
                    )
                P = ppool.tile([128, S], FP16, tag="P")
                r = rpool.tile([128, 1], FP32, tag="r")
                rinv = rpool.tile([128, 1], FP32, tag="rinv")
                nc.scalar.activation(
                    out=P,
                    in_=sc,
                    func=mybir.ActivationFunctionType.Exp,
                    scale=SCALE,
                    accum_out=r,
                )
                nc.vector.reciprocal(out=rinv, in_=r)
                if b == 0:
                    nc.vector.tensor_scalar_mul(
                        out=A[:, h, it, :], in0=P, scalar1=rinv
                    )
                else:
                    Pw = ppool.tile([128, S], FP16, tag="Pw", bufs=8)
                    nc.vector.tensor_scalar_mul(out=Pw, in0=P, scalar1=rinv)
                    if b == 1 and it % 2 == 0:
                        nc.gpsimd.tensor_add(A[:, h, it, :], A[:, h, it, :], Pw)
                    else:
                        nc.vector.tensor_add(A[:, h, it, :], A[:, h, it, :], Pw)

            def atrans_block(h, it, evac):
                aps = tp.tile([128, 1024], FP16, tag="tp", name="aps")
                for jt in range(8):
                    nc.tensor.matmul(
                        aps[:, ts(jt, 128)],
                        lhsT=A[:, h, it, ts(jt, 128)],
                        rhs=ident,
                        is_transpose=True,
                        start=(jt == 0),
                        stop=(jt == 7),
                    )
                aps3 = aps.rearrange("c (jt p) -> c jt p", jt=8)
                if evac == "v":
                    nc.vector.tensor_copy(AT[:, h, :, ts(it, 128)], aps3)
                else:
                    nc.scalar.copy(AT[:, h, :, ts(it, 128)], aps3)

            ovT = [
                ovpool.tile([C, S], FP16, tag="ovT", name=f"ovT{b}")
                for b in range(B)
            ]

            def av_chunk(h, p, n):
                av = pp_qkv.tile([128, 512], FP32, tag="ps", name="av")
                for jt in range(8):
                    nc.tensor.matmul(
                        av,
                        lhsT=vpair[:, h, p, jt, :],
                        rhs=AT[:, h, jt, ts(n, 512)],
                        start=(jt == 0),
                        stop=(jt == 7),
                    )
                for half in range(2):
                    b = 2 * p + half
                    dst = ovT[b][h * 64 : h * 64 + 64, ts(n, 512)]
                    src = av[half * 64 : half * 64 + 64, :]
                    if half == 0:
                        nc.scalar.copy(dst, src)
                    else:
                        nc.vector.tensor_copy(dst, src)

            def outproj_chunk(b, st):
                # alternate psum rings so four slots cycle through the tail
                o_sb = opool.tile([128, D], FP16, tag="osb")
                if st % 2 == 0:
                    o_ps = pp_sc.tile([128, D], FP32, tag="sc", name="ops")
                    for n in range(2):
                        nc.tensor.matmul(
                            o_ps[:, ts(n, 512)],
                            lhsT=ovT[b][:, ts(st, 128)],
                            rhs=wo_sb[:, ts(n, 512)],
                            start=True,
                            stop=True,
                        )
                    nc.scalar.copy(o_sb, o_ps)
                else:
                    for n in range(2):
                        o_ph = pp_qkv.tile(
                            [128, 512], FP32, tag="ps", name="oph"
                        )
                        nc.tensor.matmul(
                            o_ph,
                            lhsT=ovT[b][:, ts(st, 128)],
                            rhs=wo_sb[:, ts(n, 512)],
                            start=True,
                            stop=True,
                        )
                        nc.vector.tensor_copy(o_sb[:, ts(n, 512)], o_ph)
                nc.sync.dma_start(
                    out=out[b * S + st * 128 : b * S + (st + 1) * 128, :],
                    in_=o_sb,
                )

            for b in range(B):
                side = []
                if b == 0:
                    def load_wo():
                        nc.sync.dma_start(out=wo_sb, in_=woT)
                    side.append(load_wo)
                if b > 0:
                    # this quarter's v-proj first: its data is resident, so
                    # the PE never head-of-line blocks on the fresh quarter
                    # b+1 DMA at early its (the batch-boundary HAM stall)
                    for n2 in range(2):
                        side.append(
                            lambda n2=n2, xq=xq, b=b: proj_group(
                                xq, 2, vT, b, n2
                            )
                        )
                    side.append(lambda b=b: vtrans_group(b))
                if b < B - 1:
                    yq2 = load_quarter(yT, b + 1, "xyq")
                    xq2 = load_quarter(xT, b + 1, "xyq")
                    for n2 in range(2):
                        side.append(
                            lambda n2=n2, yq2=yq2, b=b: proj_group(
                                yq2, 0, qT, b + 1, n2
                            )
                        )
                    for n2 in range(2):
                        side.append(
                            lambda n2=n2, xq2=xq2, b=b: proj_group(
                                xq2, 1, kT, b + 1, n2
                            )
                        )
                if b == 0:
                    for n2 in range(2):
                        side.append(
                            lambda n2=n2, xq=xq, b=b: proj_group(
                                xq, 2, vT, b, n2
                            )
                        )
                    side.append(lambda b=b: vtrans_group(b))

                if b < B - 1:
                    for it in range(S // 128):
                        if it < len(side):
                            side[it]()
                        for h in range(2):
                            softmax_block(b, h, it)
                    xq = xq2
                else:
                    for it in range(S // 128):
                        if it < len(side):
                            side[it]()
                        softmax_block(b, 0, it)
                        if it >= 2:
                            atrans_block(0, it - 2, "v" if it % 2 else "s")
                    for it in range(S // 128):
                        softmax_block(b, 1, it)
                        if it == 0:
                            atrans_block(0, 6, "s")
                        elif it == 1:
                            atrans_block(0, 7, "v")
                        else:
                            atrans_block(1, it - 2, "v" if it % 2 else "s")
                        if it == 3:
                            av_chunk(0, 0, 0)
                        elif it == 5:
                            av_chunk(0, 0, 1)
                        elif it == 6:
                            av_chunk(0, 1, 0)
                        elif it == 7:
                            av_chunk(0, 1, 1)
                    atrans_block(1, 6, "s")
                    atrans_block(1, 7, "v")

            av_chunk(1, 0, 0)
            av_chunk(1, 0, 1)
            av_chunk(1, 1, 0)
            for st in range(4):
                outproj_chunk(0, st)
            av_chunk(1, 1, 1)
            for st in range(4, 8):
                outproj_chunk(0, st)
            for b in range(1, B):
                for st in range(8):
                    outproj_chunk(b, st)

    return nc


_PROGRAM = None


def _get_program():
    global _PROGRAM
    if _PROGRAM is None:
        _PROGRAM = build_program()
        _PROGRAM.finalize()
    return _PROGRAM


def _host_in_maps(x, y, Wq, Wk, Wv, Wo, bq, bk, bv):
    xT16 = np.ascontiguousarray(x.reshape(BS, D).T).astype(np.float16)
    yT16 = np.ascontiguousarray(y.reshape(BS, D).T).astype(np.float16)
    in_maps = []
    for c in range(NCORES):
        rows = slice(c * C, (c + 1) * C)
        wqkv = np.concatenate(
            [Wq[rows, :].T, Wk[rows, :].T, Wv[rows, :].T], axis=1
        )
        bqkv = np.stack([bq[rows], bk[rows], bv[rows]], axis=1)
        in_maps.append(
            {
                "yT": yT16,
                "xT": xT16,
                "wqkvT": np.ascontiguousarray(wqkv).astype(np.float16),
                "woT": np.ascontiguousarray(Wo[:, rows].T).astype(np.float16),
                "bqkv": np.ascontiguousarray(bqkv).astype(np.float32),
            }
        )
    return in_maps


def kernel(**inputs):
    x = np.asarray(inputs["x"], dtype=np.float32)
    y = np.asarray(inputs["y"], dtype=np.float32)
    Wq = np.asarray(inputs["Wq"], dtype=np.float32)
    Wk = np.asarray(inputs["Wk"], dtype=np.float32)
    Wv = np.asarray(inputs["Wv"], dtype=np.float32)
    Wo = np.asarray(inputs["Wo"], dtype=np.float32)
    bq = np.asarray(inputs["bq"], dtype=np.float32)
    bk = np.asarray(inputs["bk"], dtype=np.float32)
    bv = np.asarray(inputs["bv"], dtype=np.float32)
    bo = np.asarray(inputs["bo"], dtype=np.float32)

    in_maps = _host_in_maps(x, y, Wq, Wk, Wv, Wo, bq, bk, bv)
    nc = _get_program()
    res = run_bass_kernel_spmd(nc, in_maps, list(range(NCORES)))

    acc = np.zeros((BS, D), dtype=np.float32)
    for c in range(NCORES):
        acc += res.results[c]["out"].astype(np.float32)
    acc += bo[None, :]
    return acc.reshape(B, S, D)
